# revision 89
# baseline (speedup 1.0000x reference)
"""Trainium2 Bass kernel for nn_CacaAttention (GQA + RoPE + sliding-window SDPA).

Sharding (8 cores, head tensor-parallel per the hint):
  - core c gets q-heads {2c, 2c+1} (w_q cols), its KV head c//2 (w_k/w_v cols,
    replicated x2 since KV-head groups stay intact), and the matching w_o rows.
  - hidden_states is replicated (projections contract over the full model dim),
    pre-transposed to [HID, S] and cast to bf16 on the host.
  - each core emits a partial o_proj output [S, HID] in bf16; the host upcasts
    and sums the 8 partials (the gather for contraction-dim tensor parallelism).

Per-core kernel (all matmuls bf16, fp32 PSUM accumulation), fully software-
pipelined so the PE (the bottleneck engine) never idles:
  A) QKV projections straight from the host-transposed hsT (no on-chip hidden
     transpose); RoPE with the rotate-half partition swap done on the PE via a
     permutation matmul (no DMA round-trip); V moved to natural [token, d]
     layout with one DMA-XBAR transpose per 512-token block.
  B) attention in transposed-score layout S^T=[k,q] with BOTH q-heads fused
     into every matmul (they share the GQA KV head, so K/V stationary tiles
     serve a [2h, 256q] moving side); exp on the Activation engine writes
     probabilities directly in bf16; softmax denominator via a ones-matmul
     accumulated across k-tiles (broadcast across partitions for free);
     o_proj per q-block with PSUM->SBUF bf16 cast and one store DMA per
     128-token row tile.
  The emission schedule interleaves projection blocks with the attention of
  q-blocks whose window is already resident (sliding-window attention only
  looks back), runs scores/exp one pipeline stage ahead of denominator/PV,
  spreads o_proj tiles through the stream, and weaves warm-up matmuls into
  the DMA-bound startup to keep the PE p-state at full clock.
"""
import os
import sys

sys.path.insert(0, "/opt/trn_rl_repo")
import numpy as np
import ml_dtypes

BF16 = ml_dtypes.bfloat16

# Problem constants (hardcoded per contract).
B, S, HID = 1, 2048, 2048
NH, NKV, HD = 16, 4, 128
WIN = 1024
THETA = 10000.0
NCORES = 8
HPC = NH // NCORES          # q heads per core
QC = HPC * HD               # q proj cols per core
KC = HID // 128             # contraction chunks
TB = 512                    # projection token block
NTB = S // TB
QB = 256                    # attention query block
NQB = S // QB
NKT = S // 128              # k tiles

_cache = {}


def _rope_tables():
    """cos/sin tables in transposed layout [HD, S]; sin has the rotate-half
    sign folded in (rows 0:63 negated) and is pre-swapped by 64 partitions so
    the swap can happen AFTER the elementwise multiply."""
    inv_freq = 1.0 / (THETA ** (np.arange(0, HD, 2, dtype=np.float32) / HD))
    t = np.arange(S, dtype=np.float32)
    freqs = np.outer(t, inv_freq).astype(np.float32)          # [S, HD/2]
    emb = np.concatenate((freqs, freqs), axis=-1)             # [S, HD]
    cos_t = np.cos(emb).T.astype(np.float32).copy()           # [HD, S]
    sin_t = np.sin(emb).T.astype(np.float32).copy()
    sin_t[: HD // 2] *= -1.0
    sin_sw = np.roll(sin_t, -HD // 2, axis=0).copy()
    return cos_t, sin_t, sin_sw


def _mask_bias(delta):
    """Additive bias tile [128(k-part), QB(q-free)]: 0 where
    0 <= (delta + qf - kp) <= WIN else -1e9."""
    kp = np.arange(128)[:, None]
    qf = np.arange(QB)[None, :]
    dist = delta + qf - kp
    bad = (dist < 0) | (dist > WIN)
    return np.where(bad, np.float32(-1e9), np.float32(0.0)).astype(np.float32)


def _build(niter=1):
    import concourse.bacc as bacc
    import concourse.mybir as mybir
    import concourse.tile as tile

    F32 = mybir.dt.float32
    BF = mybir.dt.bfloat16

    nc = bacc.Bacc("TRN2", target_bir_lowering=False, debug=False)

    hst = nc.dram_tensor("hst", [HID, S], BF, kind="ExternalInput").ap()
    wqkv = nc.dram_tensor("wqkv", [HID, QC + 2 * HD], BF, kind="ExternalInput").ap()
    wo = nc.dram_tensor("wo", [QC, HID], BF, kind="ExternalInput").ap()
    out = nc.dram_tensor("out", [S, HID], BF, kind="ExternalOutput").ap()

    cos_np, sin_np, _sinsw_np = _rope_tables()
    cos_c = nc.inline_tensor(cos_np.astype(BF16), "cos_c").ap()
    sin_c = nc.inline_tensor(sin_np.astype(BF16), "sin_c").ap()
    # partition-swap permutation: perm[k, m] = 1 iff k == (m + 64) % 128, so
    # lhsT=perm gives out[m, :] = in[(m + 64) % 128, :]
    perm_np = np.zeros((128, 128), dtype=BF16)
    perm_np[(np.arange(128) + 64) % 128, np.arange(128)] = 1
    perm_c = nc.inline_tensor(perm_np, "perm_c").ap()
    ident_c = nc.inline_tensor(np.eye(128, dtype=BF16), "ident_c").ap()
    # per-k-tile mask biases duplicated over the head axis [128, 2(h), QB]:
    # causal masks for the last two k-tiles (delta 0, -128) and window-edge
    # masks for the first two k-tiles when q0 >= WIN (delta WIN, WIN-128)
    mask_c = {}
    for key, delta in (("c0", 0), ("c1", -128), ("w0", WIN), ("w1", WIN - 128)):
        m = _mask_bias(delta)
        mask_c[key] = nc.inline_tensor(
            np.concatenate([m, m], axis=1), f"mask_{key}").ap()

    with nc.allow_low_precision("bf16 attention kernel, tolerance 2e-2"), \
         tile.TileContext(nc) as tc:
        with tc.tile_pool(name="consts", bufs=1) as consts, \
             tc.tile_pool(name="wpool", bufs=1) as wpool, \
             tc.tile_pool(name="hpool", bufs=1) as hpool, \
             tc.tile_pool(name="persist", bufs=1) as persist:
            ones128 = consts.tile([128, 128], BF)
            nc.vector.memset(ones128, 1.0)
            permT = consts.tile([128, 128], BF)
            identT = consts.tile([128, 128], BF)
            cosT = consts.tile([128, S], BF)
            sinE = consts.tile([128, S], BF)
            masks = {}
            for key in ("c0", "c1", "w0", "w1"):
                m = consts.tile([128, 2 * QB], F32, tag=f"mask_{key}")
                masks[key] = m

            wqkv_s = wpool.tile([128, KC, QC + 2 * HD], BF)
            wo_s = wpool.tile([128, HPC, HID], BF)
            hsT = hpool.tile([128, KC, S], BF)

            # per-block persistent activations (separate tiles avoid false
            # whole-tensor dependencies between phases)
            QTb = [persist.tile([128, HPC, TB], BF, tag=f"qt{b}", name=f"qt{b}")
                   for b in range(NTB)]
            KTb = [persist.tile([128, TB], BF, tag=f"kt{b}", name=f"kt{b}")
                   for b in range(NTB)]
            Vnb = [persist.tile([128, TB // 128, HD], BF, tag=f"vn{b}", name=f"vn{b}")
                   for b in range(NTB)]
            ATq = [persist.tile([128, HPC * QB], BF, tag=f"at{q}", name=f"at{q}")
                   for q in range(NQB)]

            def load_weights_early():
                # interleave weight and hsT-block-0 loads with growing chunk
                # sizes; DMA issue is HWDGE-bound (~650ns each) so keep the
                # count low while letting the PE start ~3us in
                chunks = [(0, 1), (1, 2), (2, 4), (4, 7), (7, 10), (10, 13), (13, 16)]
                for c0, c1 in chunks:
                    r0, r1 = c0 * 128, c1 * 128
                    nc.sync.dma_start(
                        out=wqkv_s[:, c0:c1, :],
                        in_=wqkv[r0:r1, :].rearrange("(kc p) m -> p kc m", p=128))
                    nc.sync.dma_start(
                        out=hsT[:, c0:c1, 0:TB],
                        in_=hst[r0:r1, 0:TB].rearrange("(kc p) t -> p kc t", p=128))

            def load_tables():
                nc.sync.dma_start(out=cosT, in_=cos_c)
                nc.sync.dma_start(out=sinE, in_=sin_c)
                nc.sync.dma_start(out=permT, in_=perm_c)
                nc.sync.dma_start(out=identT, in_=ident_c)

            def load_hs_block(bi, halves=False):
                t0 = bi * TB
                for k0, k1 in ((0, 8), (8, 16)) if halves else ((0, 16),):
                    nc.sync.dma_start(
                        out=hsT[:, k0:k1, t0:t0 + TB],
                        in_=hst[k0 * 128:k1 * 128, t0:t0 + TB]
                        .rearrange("(kc p) t -> p kc t", p=128))

            def load_rest():
                nc.sync.dma_start(
                    out=wo_s, in_=wo.rearrange("(ch p) n -> p ch n", p=128))
                for key in ("c0", "c1", "w0", "w1"):
                    nc.sync.dma_start(out=masks[key], in_=mask_c[key])

            for _it in range(niter):
                _phases(nc, tc, tile, mybir, F32, BF,
                        out, wqkv_s, wo_s, hsT,
                        QTb, KTb, Vnb, ATq,
                        ones128, permT, identT, cosT, sinE, masks,
                        first=(_it == 0),
                        load_weights_early=load_weights_early,
                        load_tables=load_tables,
                        load_hs_block=load_hs_block,
                        load_rest=load_rest)

    nc.compile()
    return nc


def _phases(nc, tc, tile, mybir, F32, BF,
            out, wqkv_s, wo_s, hsT, QTb, KTb, Vnb, ATq,
            ones128, permT, identT, cosT, sinE, masks,
            first, load_weights_early, load_tables, load_hs_block, load_rest):
    inv_sqrt_d = 1.0 / float(np.sqrt(HD))

    def kt_slice(kt):
        return KTb[kt // (TB // 128)][:, (kt % (TB // 128)) * 128:
                                      (kt % (TB // 128)) * 128 + 128]

    def vn_slice(kt):
        return Vnb[kt // (TB // 128)][:, kt % (TB // 128), :]

    def qt2_slice(qb):
        """both heads' roped q for this q-block: [128, 2, QB]"""
        b, off = qb // (TB // QB), (qb % (TB // QB)) * QB
        return QTb[b][:, :, off:off + QB]

    def kt_range(qb):
        q0 = qb * QB
        kt_lo = max(0, (q0 - WIN) // 128)
        nkt = (q0 + QB - 1) // 128 - kt_lo + 1
        return kt_lo, nkt

    def mask_key(qb, i, kt_lo, nkt):
        if i == nkt - 1:
            return "c1"
        if i == nkt - 2:
            return "c0"
        if qb * QB >= WIN:
            if i == 0:
                return "w0"
            if i == 1:
                return "w1"
        return None

    # Single fully-interleaved phase: projection block bi feeds the attention
    # of the q-blocks it completes (sliding-window attention only looks back),
    # so the PE alternates big projection GEMM stretches with attention/o_proj
    # work while DVE ropes and the Activation engine exponentiates.  All
    # [128, 512]-f32 PSUM users (projection accumulators and o_proj tiles)
    # round-robin one 4-bank tag; scores/denominator/PV use the other 4.
    with tc.tile_pool(name="atmp", bufs=2) as atmp, \
         tc.tile_pool(name="epool", bufs=3) as epool, \
         tc.tile_pool(name="opool", bufs=4) as opool, \
         tc.tile_pool(name="rtmp", bufs=2) as rtmp, \
         tc.tile_pool(name="psum", bufs=1, space="PSUM") as psum:

        def emit_proj_block(bi, hooks=None):
            """hooks: dict kc -> tuple of thunks emitted after that kc group"""
            t0 = bi * TB
            q0p = psum.tile([128, TB], F32, tag="blk", bufs=4, name="q0p")
            q1p = psum.tile([128, TB], F32, tag="blk", bufs=4, name="q1p")
            kp_ = psum.tile([128, TB], F32, tag="blk", bufs=4, name="kp")
            vp = psum.tile([128, TB], F32, tag="blk", bufs=4, name="vp")
            for kc in range(KC):
                st, sp = (kc == 0), (kc == KC - 1)
                rhs = hsT[:, kc, t0:t0 + TB]
                nc.tensor.matmul(kp_, wqkv_s[:, kc, 256:384], rhs, start=st, stop=sp)
                nc.tensor.matmul(vp, wqkv_s[:, kc, 384:512], rhs, start=st, stop=sp)
                nc.tensor.matmul(q0p, wqkv_s[:, kc, 0:128], rhs, start=st, stop=sp)
                nc.tensor.matmul(q1p, wqkv_s[:, kc, 128:256], rhs, start=st, stop=sp)
                if hooks is not None:
                    for f in hooks.get(kc, ()):
                        f()

            # drain the psum banks fast (3 engines in parallel) so the next
            # blk-tag user isn't WAR-blocked; rope then runs all-SBUF in bf16.
            # separate tiles per tensor so readers don't wait unrelated copies
            pcs = [atmp.tile([128, TB], BF, tag=f"pc{i}", name=f"pc{i}")
                   for i in range(4)]
            nc.vector.tensor_copy(pcs[2], kp_)
            nc.scalar.copy(pcs[1], q1p)
            nc.scalar.copy(pcs[0], q0p)
            nc.vector.tensor_copy(pcs[3], vp)
            tv = psum.tile([128, 2 * QB], F32, tag="sp", bufs=3, name="tv")
            tvb = tv.bitcast(BF)[:, 0:512]
            for j in range(TB // 128):
                nc.tensor.transpose(tvb[:, j * 128:(j + 1) * 128],
                                    pcs[3][:, j * 128:(j + 1) * 128], identT)
            if bi % 2:
                nc.scalar.copy(Vnb[bi], tvb)
            else:
                nc.vector.tensor_copy(Vnb[bi], tvb)

            # rope: dst = x*cos + swap64(x)*sin_sw  (sin sign-folded; the
            # partition swap happens on the PE via permT).  Returned as a
            # closure the scheduler emits slightly later so the xs matmuls
            # never head-of-line block the PE behind the pc copies.
            def finish_rope():
                t3 = atmp.tile([128, 3, TB], BF, tag="t3")
                u3 = atmp.tile([128, 3, TB], BF, tag="u3")
                rope_dst = {0: QTb[bi][:, 0, :], 1: QTb[bi][:, 1, :], 2: KTb[bi]}
                for i in (2, 0, 1):
                    xs = psum.tile([128, 2 * QB], F32, tag="sp", bufs=3, name="xs")
                    nc.tensor.matmul(xs, permT, pcs[i], start=True, stop=True)
                    nc.vector.tensor_mul(t3[:, i, :], pcs[i], cosT[:, t0:t0 + TB])
                    nc.vector.tensor_mul(u3[:, i, :], xs, sinE[:, t0:t0 + TB])
                    nc.vector.tensor_add(rope_dst[i], t3[:, i, :], u3[:, i, :])
            return finish_rope

        def emit_score_kt(qb, E, i, kt_lo, nkt):
            """one k-tile of QK^T for both heads, mask, exp -> E[:, i]"""
            sp2 = psum.tile([128, 2 * QB], F32, tag="sp", bufs=3, name="sp2")
            nc.tensor.matmul(sp2, kt_slice(kt_lo + i), qt2_slice(qb),
                             start=True, stop=True)
            mk = mask_key(qb, i, kt_lo, nkt)
            if mk is not None:
                nc.vector.tensor_add(sp2, sp2, masks[mk])
            nc.scalar.activation(E[:, i, :], sp2,
                                 mybir.ActivationFunctionType.Exp,
                                 scale=inv_sqrt_d)

        def oproj_unit(qb, ts, cg, osts, fine=False):
            """one [128,512] o_proj tile: 2 matmuls + psum->bf16 copy (+DMA
            after the row-tile's last column group; half-row DMAs when fine)"""
            if cg == 0:
                osts[ts] = opool.tile([128, HID], BF, tag="ost", name="ost")
            ost = osts[ts]
            op = psum.tile([128, 512], F32, tag="blk", bufs=4, name="op")
            for ch in range(HPC):
                nc.tensor.matmul(
                    op, ATq[qb][:, ch * QB + ts * 128:ch * QB + (ts + 1) * 128],
                    wo_s[:, ch, cg * 512:(cg + 1) * 512],
                    start=(ch == 0), stop=(ch == HPC - 1))
            dst = ost[:, cg * 512:(cg + 1) * 512]
            if (ts * (HID // 512) + cg) % 2:
                nc.scalar.copy(dst, op)
            else:
                nc.vector.tensor_copy(dst, op)
            trow = qb * QB + ts * 128
            if fine and cg % 2 == 1:
                half = (cg // 2) * 1024
                nc.sync.dma_start(
                    out=out[trow:trow + 128, half:half + 1024],
                    in_=ost[:, half:half + 1024])
            elif not fine and cg == HID // 512 - 1:
                nc.sync.dma_start(out=out[trow:trow + 128, :], in_=ost)

        def oproj_units(qb, fine=False):
            osts = {}
            return [lambda ts=ts, cg=cg: oproj_unit(qb, ts, cg, osts, fine)
                    for ts in range(QB // 128) for cg in range(HID // 512)]

        warm = psum.tile([128, 2 * QB], F32, tag="pv", bufs=1, name="warm")

        def emit_warm(n):
            for _ in range(n):
                nc.tensor.matmul(warm[:, 0:128], ones128, ones128,
                                 start=True, stop=True)

        if first:
            # warm-up matmuls on an already-memset const: fill the initial
            # DMA-supply stalls and ramp the PE p-state to full clock; more
            # are woven between P0's kc groups where the DMA train lags
            emit_warm(16)
            load_weights_early()
            load_hs_block(1, halves=True)
            load_tables()
            load_hs_block(2)
            load_rest()
            load_hs_block(3)
        else:
            for bi in range(NTB):
                load_hs_block(bi)

        # Software pipeline, one stage deep: while the Activation engine
        # exponentiates q-block qb's scores, the PE runs the previous A's
        # denominator/PV matmuls (interleaved per k-tile so the PE never
        # outruns the exp stream on the sp psum buffers) and an older A's
        # o_proj.  qb0 (tiny) is deferred to the end to shorten the drain.
        state = {"prevA": None, "oproj_q": []}

        def run_A(qb):
            if qb is not None:
                kt_lo, nkt = kt_range(qb)
                E = epool.tile([128, WIN // 128 + QB // 128, 2 * QB], BF,
                               tag="E", name="E")
            else:
                nkt = 0
            prevA = state["prevA"]
            if prevA is not None:
                pq, Ep, plo, pn = prevA
                dn2 = psum.tile([128, 2 * QB], F32, tag="blk", bufs=4, name="dn2")
                pv2 = psum.tile([128, 2 * QB], F32, tag="pv", bufs=1, name="pv2")
            else:
                pn = 0
            oq = state["oproj_q"]
            keep = 1 if qb is not None else 0
            units = []
            while len(oq) > keep:
                units.extend(oproj_units(oq.pop(0)))
            # the dn stream lags the scores (so it never WAR-blocks on psum
            # buffers still being drained) and leads the pv stream, letting
            # the reciprocal run on DVE while the PE still streams PV
            lag_dn = 1 if (qb is not None and prevA is not None) else 0
            lag_pv = lag_dn + 2
            n_iter = max(nkt, pn + lag_pv, 1)
            # hold a few o_proj units back to cover the reciprocal/AT latency
            denom = n_iter * 100 if qb is None else n_iter
            ui = 0
            rec2 = None
            for i in range(n_iter):
                if qb is not None and i < nkt:
                    emit_score_kt(qb, E, i, kt_lo, nkt)
                if i == 0 and state.get("hook") is not None:
                    state.pop("hook")()
                j = i - lag_dn
                if prevA is not None and 0 <= j < pn:
                    nc.tensor.matmul(dn2, ones128, Ep[:, j, :],
                                     start=(j == 0), stop=(j == pn - 1))
                    if j == pn - 1:
                        rec2 = rtmp.tile([128, 2 * QB], BF, tag="rec")
                        nc.vector.reciprocal(rec2, dn2)
                j = i - lag_pv
                if prevA is not None and 0 <= j < pn:
                    nc.tensor.matmul(pv2, vn_slice(plo + j), Ep[:, j, :],
                                     start=(j == 0), stop=(j == pn - 1))
                quota = (i * len(units)) // max(denom - 1, 1)
                while ui < quota:
                    units[ui]()
                    ui += 1
            if prevA is not None:
                nc.vector.tensor_mul(ATq[pq], pv2, rec2)
                oq.append(pq)
            while ui < len(units):
                units[ui]()
                ui += 1
            if qb is None:
                while oq:
                    fine = len(oq) == 1
                    for u in oproj_units(oq.pop(0), fine=fine):
                        u()
            state["prevA"] = (qb, E, kt_lo, nkt) if qb is not None else None

        rope0 = emit_proj_block(
            0, None if not first else None)
        rope1 = emit_proj_block(1, {1: (rope0,)})
        # qb1's scores wedge into P2's kc loop so its exp stream overlaps the
        # projection GEMMs instead of stalling the in-order PE queue
        kt_lo1, nkt1 = kt_range(1)
        E1 = epool.tile([128, WIN // 128 + QB // 128, 2 * QB], BF,
                        tag="E", name="E1")
        rope2 = emit_proj_block(2, {
            1: (rope1,),
            3: (lambda: emit_score_kt(1, E1, 0, kt_lo1, nkt1),
                lambda: emit_score_kt(1, E1, 1, kt_lo1, nkt1)),
            7: (lambda: emit_score_kt(1, E1, 2, kt_lo1, nkt1),),
            11: (lambda: emit_score_kt(1, E1, 3, kt_lo1, nkt1),)})
        state["prevA"] = (1, E1, kt_lo1, nkt1)
        state["hook"] = rope2
        run_A(2)
        run_A(3)
        rope3 = emit_proj_block(3)
        state["hook"] = rope3
        for qb in (4, 5, 6, 0, 7, None):
            run_A(qb)


def _get_nc(niter=1):
    key = f"nc{niter}"
    if key not in _cache:
        _cache[key] = _build(niter)
    return _cache[key]


def _shard_inputs(hidden_states, w_q, w_k, w_v, w_o):
    hs = np.asarray(hidden_states, dtype=np.float32).reshape(S, HID)
    hst = np.ascontiguousarray(hs.T).astype(BF16)
    w_q = np.asarray(w_q, dtype=np.float32)
    w_k = np.asarray(w_k, dtype=np.float32)
    w_v = np.asarray(w_v, dtype=np.float32)
    w_o = np.asarray(w_o, dtype=np.float32)
    in_maps = []
    for c in range(NCORES):
        kvh = c // (NCORES // NKV)
        wqkv = np.concatenate([w_q[:, c * QC:(c + 1) * QC],
                               w_k[:, kvh * HD:(kvh + 1) * HD],
                               w_v[:, kvh * HD:(kvh + 1) * HD]], axis=1)
        in_maps.append({
            "hst": hst,
            "wqkv": np.ascontiguousarray(wqkv).astype(BF16),
            "wo": np.ascontiguousarray(w_o[c * QC:(c + 1) * QC, :]).astype(BF16),
        })
    return in_maps


def _get_runner(niter=1):
    """Jitted 8-core executor with device-resident zero-out buffers (no
    donation, so repeated timed calls don't re-upload)."""
    rkey = ("runner", niter)
    if rkey in _cache:
        return _cache[rkey]
    import jax
    import concourse.mybir as mybir
    from jax.sharding import Mesh, PartitionSpec
    from jax.experimental.shard_map import shard_map
    from concourse.bass2jax import (
        _bass_exec_p, install_neuronx_cc_hook, partition_id_tensor)

    install_neuronx_cc_hook()
    nc = _get_nc(niter)
    pname = nc.partition_id_tensor.name if nc.partition_id_tensor else None

    in_names, out_names, out_avals = [], [], []
    for alloc in nc.m.functions[0].allocations:
        if not isinstance(alloc, mybir.MemoryLocationSet):
            continue
        name = alloc.memorylocations[0].name
        if alloc.kind == "ExternalInput":
            if name != pname:
                in_names.append(name)
        elif alloc.kind == "ExternalOutput":
            out_names.append(name)
            out_avals.append(jax.core.ShapedArray(
                tuple(alloc.tensor_shape), mybir.dt.np(alloc.dtype)))
    n_params = len(in_names)
    all_names = in_names + out_names
    if pname is not None:
        all_names = all_names + [pname]

    def _body(*args):
        operands = list(args)
        if pname is not None:
            operands.append(partition_id_tensor())
        outs = _bass_exec_p.bind(
            *operands,
            out_avals=tuple(out_avals),
            in_names=tuple(all_names),
            out_names=tuple(out_names),
            lowering_input_output_aliases=(),
            sim_require_finite=True,
            sim_require_nnan=True,
            nc=nc,
        )
        return tuple(outs)

    devices = jax.devices()[:NCORES]
    mesh = Mesh(np.asarray(devices), ("core",))
    nspec = n_params + len(out_names)
    fn = jax.jit(shard_map(
        _body, mesh=mesh,
        in_specs=(PartitionSpec("core"),) * nspec,
        out_specs=(PartitionSpec("core"),) * len(out_names),
        check_rep=False))
    _cache[rkey] = (fn, in_names, out_names, out_avals)
    return _cache[rkey]


def _prep_device_args(in_maps):
    import jax
    fn, in_names, out_names, out_avals = _get_runner()
    concat_in = [np.concatenate([np.asarray(in_maps[c][n]) for c in range(NCORES)], axis=0)
                 for n in in_names]
    zeros = [np.zeros((NCORES * a.shape[0], *a.shape[1:]), a.dtype) for a in out_avals]
    return [jax.device_put(x) for x in concat_in + zeros]


def _run(in_maps):
    fn, in_names, out_names, out_avals = _get_runner()
    args = _prep_device_args(in_maps)
    outs = fn(*args)
    _cache["last_args"] = args
    return [
        {n: np.asarray(outs[i]).reshape(NCORES, *out_avals[i].shape)[c]
         for i, n in enumerate(out_names)}
        for c in range(NCORES)
    ]


def time_kernel(reps=10, n=16, m=16):
    """Marginal per-kernel-iteration device time (ns): pipelined loops of m
    dispatches of an n-iteration-unrolled build vs the 1-iteration build.
    Dispatch overhead (~31ms/call, pipelined) cancels in the difference.
    Noisy on this axon setup — treat as a rough cross-check of the
    cost-model (TimelineSim) estimate."""
    import time
    args = _cache.get("last_args")
    assert args is not None, "run kernel() first"

    def timed(niter):
        fn, _, _, _ = _get_runner(niter)
        for o in fn(*args):
            o.block_until_ready()  # warm/compile
        ts = []
        for _ in range(reps):
            t0 = time.perf_counter()
            outs = None
            for _ in range(m):
                outs = fn(*args)
            for o in outs:
                o.block_until_ready()
            ts.append((time.perf_counter() - t0) / m)
        return ts

    t1 = sorted(timed(1))
    tn = sorted(timed(n))
    print(f"  niter=1 : " + " ".join(f"{t*1e3:.2f}" for t in t1), flush=True)
    print(f"  niter={n}: " + " ".join(f"{t*1e3:.2f}" for t in tn), flush=True)
    k = max(2, reps // 3)
    est = (sum(tn[:k]) / k - sum(t1[:k]) / k) / (n - 1) * 1e9
    return est


def kernel(hidden_states, w_q, w_k, w_v, w_o):
    in_maps = _shard_inputs(hidden_states, w_q, w_k, w_v, w_o)
    results = _run(in_maps)
    acc = np.zeros((S, HID), dtype=np.float32)
    for c in range(NCORES):
        acc += results[c]["out"].astype(np.float32)
    return acc.reshape(B, S, HID)
